# revision 12
# baseline (speedup 1.0000x reference)
"""Trainium2 Bass kernel for DepthwiseXCorr (SiamRPN-style head).

Pipeline per batch sample:
  k = BN+ReLU(conv1x1(kernel, w_k))      [256, 7, 7]
  s = BN+ReLU(conv1x1(search, w_s))      [256, 31, 31]
  feat = depthwise_xcorr(s, k)           [256, 25, 25]
  y = BN+ReLU(conv1x1(feat, w_h1))
  out = conv1x1(y, w_h2) + bias          [20, 25, 25]

Sharding: pure data-parallel, batch 128 -> 16 per core across 8 cores.

Implementation notes:
- conv1x1 = matmul with channels on the contraction (partition) dim.
- depthwise xcorr: the 49 kernel taps (u, v) are split across FOUR engines:
  * P_TAPS on the PE as diag(k[:, u, v]) @ s[:, i+u, j+v] matmuls
    accumulating in PSUM; the diagonal weight tiles are built by the Pool
    (GPSIMD) engine with affine_select (iota p-j == 0 selects a stride-0
    broadcast of the per-channel scalar), which is far cheaper than
    scaling a 128x128 identity on DVE/ScalarE.
  * D_FULL taps run fully on VectorE (4x-mode tensor_scalar mult +
    2x-mode tensor_tensor add into a bf16 accumulator acc_d).
  * A_TAPS taps: ScalarE does the multiply (activation Copy with
    per-partition scale), VectorE adds into acc_d.
  * M_TAPS taps: VectorE does the multiply, Pool adds into acc_p.
  Pool then merges acc_d into acc_p, and one identity-weight matmul per
  PSUM phase folds acc_p into the PSUM accumulation group.
- Shifted search windows are plain strided APs into the search feature
  tile - no data movement.
- All matmuls in bf16 (PE full column rate), accumulation in fp32 PSUM,
  BN+ReLU epilogues on ScalarE with fp32 math, fp32 output.
"""

import sys

if "/opt/trn_rl_repo" not in sys.path:
    sys.path.insert(0, "/opt/trn_rl_repo")

import ml_dtypes
import numpy as np

B, CIN, HID, COUT = 128, 256, 256, 20
NCORES = 8
NB = B // NCORES          # batches per core
HS = 31                   # search spatial
HK = 7                    # kernel spatial
HO = HS - HK + 1          # 25, xcorr output spatial
EPS = 1e-5
GB = 4                    # batch group size for the search-branch pipeline
NCH = 2                   # channel chunks of 128

# xcorr tap assignment (49 taps total)
P_TAPS = 25               # PE diag matmuls (diags built on Pool)
D_FULL = 6                # DVE mult + DVE add -> acc_d
A_TAPS = 7                # ScalarE mult + DVE add -> acc_d
M_TAPS = 11               # DVE mult + Pool add -> acc_p
assert P_TAPS + D_FULL + A_TAPS + M_TAPS == 49

BF16 = ml_dtypes.bfloat16

_CACHE = {}


def _build_nc(repeat=1):
    import concourse.bacc as bacc
    import concourse.tile as tile
    from concourse import mybir

    f32 = mybir.dt.float32
    bf16 = mybir.dt.bfloat16

    nc = bacc.Bacc()

    xk = nc.dram_tensor("xk", [NB, CIN, HK, HK], bf16, kind="ExternalInput")
    xs = nc.dram_tensor("xs", [NB, CIN, HS, HS], bf16, kind="ExternalInput")
    wkT = nc.dram_tensor("wkT", [CIN, HID], bf16, kind="ExternalInput")
    wsT = nc.dram_tensor("wsT", [CIN, HID], bf16, kind="ExternalInput")
    wh1T = nc.dram_tensor("wh1T", [HID, HID], bf16, kind="ExternalInput")
    wh2T = nc.dram_tensor("wh2T", [HID, COUT], bf16, kind="ExternalInput")
    # packed per-channel scalars: cols = sck,shk,scs,shs,sch,shh,bh2(pad)
    scl = nc.dram_tensor("scl", [HID, 8], f32, kind="ExternalInput")
    ident = nc.dram_tensor("ident", [128, 128], bf16, kind="ExternalInput")
    out = nc.dram_tensor("out", [NB, COUT, HO, HO], f32, kind="ExternalOutput")

    relu = mybir.ActivationFunctionType.Relu
    idfn = mybir.ActivationFunctionType.Identity
    copyfn = mybir.ActivationFunctionType.Copy
    mult = mybir.AluOpType.mult
    addop = mybir.AluOpType.add
    iseq = mybir.AluOpType.is_equal

    with tile.TileContext(nc) as tc:
        with (
            tc.tile_pool(name="const", bufs=1) as cpool,
            tc.tile_pool(name="act", bufs=1) as apool,
            tc.tile_pool(name="stream", bufs=2) as spool,
            tc.tile_pool(name="diagp", bufs=3 * P_TAPS) as dpool,
            tc.tile_pool(name="psum", bufs=1, space="PSUM") as ppool,
        ):
            # ---- constants -------------------------------------------------
            # critical-path loads first: the kernel branch gates the xcorr
            # diag builds, so wk/scl/xk go ahead; ws next (search conv);
            # heads + ident later. Scale vectors come packed in one tensor.
            wk_t, ws_t, wh1_t, wh2_t = [], [], [], []
            scl_t = []
            for kc in range(NCH):
                sl = slice(kc * 128, (kc + 1) * 128)
                w1 = cpool.tile([128, HID], bf16, name=f"wk_{kc}")
                nc.sync.dma_start(w1[:], wkT[sl, :])
                wk_t.append(w1)
                sv = cpool.tile([128, 8], f32, name=f"scl_{kc}")
                nc.sync.dma_start(sv[:], scl[sl, :])
                scl_t.append(sv)
            xk_ap = xk[:].rearrange("b c h w -> c b (h w)")
            xk_t = []
            for kc in range(NCH):
                t = apool.tile([128, NB, HK * HK], bf16, name=f"xk_t{kc}")
                nc.sync.dma_start(t[:], xk_ap[kc * 128:(kc + 1) * 128])
                xk_t.append(t)
            for kc in range(NCH):
                sl = slice(kc * 128, (kc + 1) * 128)
                w2 = cpool.tile([128, HID], bf16, name=f"ws_{kc}")
                nc.sync.dma_start(w2[:], wsT[sl, :])
                ws_t.append(w2)
            for kc in range(NCH):
                sl = slice(kc * 128, (kc + 1) * 128)
                w3 = cpool.tile([128, HID], bf16, name=f"wh1_{kc}")
                nc.sync.dma_start(w3[:], wh1T[sl, :])
                wh1_t.append(w3)
                w4 = cpool.tile([128, COUT], bf16, name=f"wh2_{kc}")
                nc.sync.dma_start(w4[:], wh2T[sl, :])
                wh2_t.append(w4)
            id_t = cpool.tile([128, 128], bf16, name="id_t")
            nc.sync.dma_start(id_t[:], ident[:])
            sck_t = [sv[:, 0:1] for sv in scl_t]
            shk_t = [sv[:, 1:2] for sv in scl_t]
            scs_t = [sv[:, 2:3] for sv in scl_t]
            shs_t = [sv[:, 3:4] for sv in scl_t]
            sch_t = [sv[:, 4:5] for sv in scl_t]
            shh_t = [sv[:, 5:6] for sv in scl_t]
            bh2_t = scl_t[0][0:COUT, 6:7]

            # ---- kernel branch conv (all NB batches at once) ---------------
            # k_feat[mc]: [128, NB, 49] fp32 (source of per-partition scalars)
            k_feat = []
            for mc in range(NCH):
                kf = apool.tile([128, NB, HK * HK], f32, name=f"k_feat{mc}")
                for half in range(2):
                    bs = slice(half * (NB // 2), (half + 1) * (NB // 2))
                    ps = ppool.tile([128, NB // 2, HK * HK], f32,
                                    name="ps_cs", tag="csA", bufs=1)
                    for kc in range(NCH):
                        nc.tensor.matmul(
                            ps[:],
                            wk_t[kc][:, mc * 128:(mc + 1) * 128],
                            xk_t[kc][:, bs, :],
                            start=(kc == 0), stop=(kc == NCH - 1),
                        )
                    nc.scalar.activation(kf[:, bs, :], ps[:], relu,
                                         bias=shk_t[mc], scale=sck_t[mc])
                k_feat.append(kf)

            for _rep in range(repeat):
              # ---- main loop over batch groups -----------------------------
              NGRP = NB // GB
              xs_ap = xs[:].rearrange("b c h w -> c b (h w)")
              feat = {}
              ys = {}

              def tap_split(g, bl):
                  """(pe, dve, act, pool) tap counts; the final unit leans
                  on the PE so the vector engines drain earlier."""
                  if g == NGRP - 1 and bl == GB - 1:
                      return (37, 4, 4, 4)
                  return (P_TAPS, D_FULL, A_TAPS, M_TAPS)

              def emit_diags(it):
                  """diagonal weight tiles on Pool (affine_select)."""
                  g, bl, mc = it
                  b_abs = g * GB + bl
                  lst = []
                  for uv in range(tap_split(g, bl)[0]):
                      dg = dpool.tile([128, 128], bf16, name="dg", tag="diag")
                      kcol = k_feat[mc][:, b_abs, uv:uv + 1]
                      nc.gpsimd.affine_select(
                          dg[:], kcol.broadcast_to((128, 128)),
                          pattern=[[-1, 128]], compare_op=iseq,
                          fill=0.0, base=0, channel_multiplier=1)
                      lst.append(dg)
                  return lst

              def emit_group_conv(g):
                  """xs load + search conv + BN/ReLU for one batch group."""
                  gsl = slice(g * GB, (g + 1) * GB)
                  xs_t = []
                  for kc in range(NCH):
                      t = spool.tile([128, GB, HS * HS], bf16,
                                     name=f"xs_t{kc}", tag=f"xs{kc}")
                      # group 0 loads ride the (startup-idle) Act queue so
                      # they don't serialize behind the weight loads on SP
                      q = nc.scalar if g == 0 else nc.sync
                      q.dma_start(t[:],
                                  xs_ap[kc * 128:(kc + 1) * 128, gsl])
                      xs_t.append(t)
                  s_feat = []
                  for mc in range(NCH):
                      sf = spool.tile([128, GB, HS, HS], bf16,
                                      name=f"s_feat{mc}", tag=f"sf{mc}")
                      for bl in range(GB):
                          for ph, (r0, r1) in enumerate(((0, 16), (16, 31))):
                              ps = ppool.tile([128, r1 - r0, HS], f32,
                                              name="ps_cs2",
                                              tag=("csA" if ph == 0 else "csB"),
                                              bufs=1)
                              for kc in range(NCH):
                                  nc.tensor.matmul(
                                      ps[:],
                                      ws_t[kc][:, mc * 128:(mc + 1) * 128],
                                      xs_t[kc][:, bl, r0 * HS:r1 * HS],
                                      start=(kc == 0), stop=(kc == NCH - 1),
                                  )
                              nc.scalar.activation(sf[:, bl, r0:r1, :], ps[:],
                                                   relu, bias=shs_t[mc],
                                                   scale=scs_t[mc])
                      s_feat.append(sf)
                  return s_feat

              s_feat = emit_group_conv(0)
              for g in range(NGRP):
                  s_feat_next = None
                  for bl in range(GB):
                      if bl == GB - 2 and g + 1 < NGRP:
                          # hoist the next group's conv ahead of this
                          # group's last batch so ScalarE/DVE are not
                          # starved at the group boundary
                          s_feat_next = emit_group_conv(g + 1)
                      b_abs = g * GB + bl
                      n_pe, n_d, n_a, n_m = tap_split(g, bl)
                      kcol = lambda mc, uv: k_feat[mc][:, b_abs, uv:uv + 1]
                      swin = lambda mc, uv, r0, r1: s_feat[mc][
                          :, bl, uv // 7 + r0:uv // 7 + r1,
                          uv % 7:uv % 7 + HO]
                      diags2 = [emit_diags((g, bl, mc)) for mc in range(NCH)]

                      # --- vector-engine taps: both mc chunks interleaved ---
                      # acc_d2: n_d DVE taps + n_a ScalarE-mult taps, with
                      # the adds PAIRED across mc ([128,2,HO,HO] 2x-mode TT)
                      # acc_p[mc]: n_m DVE-mult/Pool-add taps
                      acc_d2 = apool.tile([128, NCH, HO, HO], bf16,
                                          name="acc_d2", tag="accd2", bufs=5)
                      acc_p2 = [apool.tile([128, HO, HO], bf16,
                                           name="acc_p", tag=f"accp{mc}",
                                           bufs=4) for mc in range(NCH)]
                      uv = n_pe
                      # seed acc_p (Pool's adds chain the longest; start
                      # its feed first)
                      for mc in range(NCH):
                          nc.vector.tensor_scalar_mul(
                              acc_p2[mc][:], swin(mc, uv, 0, HO), kcol(mc, uv))
                      uv += 1
                      for i in range(n_m - 1):
                          for mc in range(NCH):
                              tmp = apool.tile([128, HO, HO], bf16,
                                               name="tmpp", tag="tmpp", bufs=6)
                              nc.vector.tensor_scalar_mul(
                                  tmp[:], swin(mc, uv, 0, HO), kcol(mc, uv))
                              nc.gpsimd.tensor_tensor(
                                  acc_p2[mc][:], acc_p2[mc][:], tmp[:],
                                  op=addop)
                          uv += 1
                      # seed acc_d2
                      for mc in range(NCH):
                          nc.vector.tensor_scalar_mul(
                              acc_d2[:, mc], swin(mc, uv, 0, HO), kcol(mc, uv))
                      uv += 1
                      for i in range(n_d - 1 + n_a):
                          on_act = i >= n_d - 1
                          tmp2 = apool.tile([128, NCH, HO, HO], bf16,
                                            name="tmp2", tag="tmp2", bufs=12)
                          for mc in range(NCH):
                              if on_act:
                                  nc.scalar.activation(
                                      tmp2[:, mc], swin(mc, uv, 0, HO),
                                      copyfn, scale=kcol(mc, uv))
                              else:
                                  nc.vector.tensor_scalar_mul(
                                      tmp2[:, mc], swin(mc, uv, 0, HO),
                                      kcol(mc, uv))
                          nc.vector.tensor_tensor(
                              acc_d2[:], acc_d2[:], tmp2[:], addop)
                          uv += 1
                      assert uv == 49
                      # merge acc_d2 into acc_p per chunk on Pool
                      for mc in range(NCH):
                          nc.gpsimd.tensor_tensor(
                              acc_p2[mc][:], acc_p2[mc][:], acc_d2[:, mc],
                              op=addop)

                      for mc in range(NCH):
                          diags = diags2[mc]
                          acc_p = acc_p2[mc]
                          ft = apool.tile([128, HO * HO], bf16,
                                          name=f"feat{mc}", tag=f"f{mc}",
                                          bufs=3)
                          # phase A: output rows 0:20 (N=500)
                          psA = ppool.tile([128, 20 * HO], f32,
                                           name="ps_xc", tag="xc", bufs=3)
                          for t in range(n_pe):
                              nc.tensor.matmul(
                                  psA[:], diags[t][:], swin(mc, t, 0, 20),
                                  start=(t == 0), stop=False,
                              )
                          nc.tensor.matmul(
                              psA[:], id_t[:], acc_p[:, 0:20, :],
                              start=False, stop=True,
                          )
                          nc.scalar.activation(ft[:, 0:500], psA[:], copyfn)
                          # phase B: output rows 20:25 (N=125)
                          psB = ppool.tile([128, 5 * HO], f32,
                                           name="ps_xcB", tag="xc", bufs=3)
                          for t in range(n_pe):
                              nc.tensor.matmul(
                                  psB[:], diags[t][:], swin(mc, t, 20, 25),
                                  start=(t == 0), stop=False,
                              )
                          nc.tensor.matmul(
                              psB[:], id_t[:], acc_p[:, 20:25, :],
                              start=False, stop=True,
                          )
                          nc.scalar.activation(ft[:, 500:625], psB[:], copyfn)
                          feat[mc] = ft

                          if mc < NCH - 1:
                              continue

                          # head conv1 + BN/ReLU
                          for mq in range(NCH):
                              yt = apool.tile([128, HO * HO], bf16,
                                              name=f"y{mq}", tag=f"y{mq}",
                                              bufs=2)
                              for ph, (c0, c1) in enumerate(((0, 500),
                                                            (500, 625))):
                                  ps = ppool.tile([128, c1 - c0], f32,
                                                  name="ps_hd",
                                                  tag=("hdA" if ph == 0
                                                       else "hdB"),
                                                  bufs=(2 if ph == 0 else 1))
                                  for kc in range(NCH):
                                      nc.tensor.matmul(
                                          ps[:],
                                          wh1_t[kc][:, mq * 128:(mq + 1) * 128],
                                          feat[kc][:, c0:c1],
                                          start=(kc == 0), stop=(kc == NCH - 1),
                                      )
                                  nc.scalar.activation(yt[:, c0:c1], ps[:],
                                                       relu, bias=shh_t[mq],
                                                       scale=sch_t[mq])
                              ys[mq] = yt

                          # head conv2 + bias
                          ot = apool.tile([COUT, HO * HO], f32,
                                          name="ot", tag="ot", bufs=2)
                          for ph, (c0, c1) in enumerate(((0, 500), (500, 625))):
                              ps = ppool.tile([COUT, c1 - c0], f32,
                                              name="ps_o",
                                              tag=("hdA" if ph == 0 else "hdB"),
                                              bufs=(2 if ph == 0 else 1))
                              for kc in range(NCH):
                                  nc.tensor.matmul(
                                      ps[:],
                                      wh2_t[kc][:],
                                      ys[kc][:, c0:c1],
                                      start=(kc == 0), stop=(kc == NCH - 1),
                                  )
                              nc.scalar.activation(ot[:, c0:c1], ps[:], idfn,
                                                   bias=bh2_t, scale=1.0)
                          nc.sync.dma_start(
                              out[b_abs][:].rearrange("o h w -> o (h w)"), ot[:])
                  if s_feat_next is not None:
                      s_feat = s_feat_next

    nc.compile()
    return nc


def _get_nc():
    if "nc" not in _CACHE:
        _CACHE["nc"] = _build_nc()
    return _CACHE["nc"]


def kernel(kernel, search, w_k, g_k, b_k, m_k, v_k, w_s, g_s, b_s, m_s, v_s,
           w_h1, g_h, b_h, m_h, v_h, w_h2, bias_h2):
    from concourse.bass_utils import run_bass_kernel_spmd

    def fold(g, b, m, v):
        sc = (g / np.sqrt(v + EPS)).astype(np.float32)
        sh = (b - m * sc).astype(np.float32)
        return sc.reshape(-1, 1), sh.reshape(-1, 1)

    kernel, search, w_k, w_s, w_h1, w_h2, bias_h2 = [
        np.asarray(a) for a in
        (kernel, search, w_k, w_s, w_h1, w_h2, bias_h2)]
    g_k, b_k, m_k, v_k = map(np.asarray, (g_k, b_k, m_k, v_k))
    g_s, b_s, m_s, v_s = map(np.asarray, (g_s, b_s, m_s, v_s))
    g_h, b_h, m_h, v_h = map(np.asarray, (g_h, b_h, m_h, v_h))

    sck, shk = fold(g_k, b_k, m_k, v_k)
    scs, shs = fold(g_s, b_s, m_s, v_s)
    sch, shh = fold(g_h, b_h, m_h, v_h)

    scl = np.zeros((HID, 8), dtype=np.float32)
    scl[:, 0:1], scl[:, 1:2] = sck, shk
    scl[:, 2:3], scl[:, 3:4] = scs, shs
    scl[:, 4:5], scl[:, 5:6] = sch, shh
    scl[:COUT, 6] = bias_h2.astype(np.float32).ravel()
    common = {
        "wkT": np.ascontiguousarray(w_k.T).astype(BF16),
        "wsT": np.ascontiguousarray(w_s.T).astype(BF16),
        "wh1T": np.ascontiguousarray(w_h1.T).astype(BF16),
        "wh2T": np.ascontiguousarray(w_h2.T).astype(BF16),
        "scl": scl,
        "ident": np.eye(128, dtype=BF16),
    }
    xk_all = kernel.astype(BF16)
    xs_all = search.astype(BF16)

    in_maps = []
    for i in range(NCORES):
        bs = slice(i * NB, (i + 1) * NB)
        m = dict(common)
        m["xk"] = np.ascontiguousarray(xk_all[bs])
        m["xs"] = np.ascontiguousarray(xs_all[bs])
        in_maps.append(m)

    nc = _get_nc()
    res = run_bass_kernel_spmd(nc, in_maps, core_ids=list(range(NCORES)))
    return np.concatenate([res.results[i]["out"] for i in range(NCORES)],
                          axis=0)


# revision 13
# speedup vs baseline: 1.0713x; 1.0713x over previous
"""Trainium2 Bass kernel for DepthwiseXCorr (SiamRPN-style head).

Pipeline per batch sample:
  k = BN+ReLU(conv1x1(kernel, w_k))      [256, 7, 7]
  s = BN+ReLU(conv1x1(search, w_s))      [256, 31, 31]
  feat = depthwise_xcorr(s, k)           [256, 25, 25]
  y = BN+ReLU(conv1x1(feat, w_h1))
  out = conv1x1(y, w_h2) + bias          [20, 25, 25]

Sharding: pure data-parallel, batch 128 -> 16 per core across 8 cores.

Implementation notes:
- conv1x1 = matmul with channels on the contraction (partition) dim.
- depthwise xcorr: the 49 kernel taps (u, v) are split across FOUR engines:
  * P_TAPS on the PE as diag(k[:, u, v]) @ s[:, i+u, j+v] matmuls
    accumulating in PSUM; the diagonal weight tiles are built by the Pool
    (GPSIMD) engine with affine_select (iota p-j == 0 selects a stride-0
    broadcast of the per-channel scalar), which is far cheaper than
    scaling a 128x128 identity on DVE/ScalarE.
  * D_FULL taps run fully on VectorE (4x-mode tensor_scalar mult +
    2x-mode tensor_tensor add into a bf16 accumulator acc_d).
  * A_TAPS taps: ScalarE does the multiply (activation Copy with
    per-partition scale), VectorE adds into acc_d.
  * M_TAPS taps: VectorE does the multiply, Pool adds into acc_p.
  Pool then merges acc_d into acc_p, and one identity-weight matmul per
  PSUM phase folds acc_p into the PSUM accumulation group.
- Shifted search windows are plain strided APs into the search feature
  tile - no data movement.
- All matmuls in bf16 (PE full column rate), accumulation in fp32 PSUM,
  BN+ReLU epilogues on ScalarE with fp32 math, fp32 output.
"""

import sys

if "/opt/trn_rl_repo" not in sys.path:
    sys.path.insert(0, "/opt/trn_rl_repo")

import ml_dtypes
import numpy as np

B, CIN, HID, COUT = 128, 256, 256, 20
NCORES = 8
NB = B // NCORES          # batches per core
HS = 31                   # search spatial
HK = 7                    # kernel spatial
HO = HS - HK + 1          # 25, xcorr output spatial
EPS = 1e-5
GB = 4                    # batch group size for the search-branch pipeline
NCH = 2                   # channel chunks of 128

# xcorr tap assignment (49 taps total)
P_TAPS = 25               # PE diag matmuls (diags built on Pool)
D_FULL = 6                # DVE mult + DVE add -> acc_d
A_TAPS = 7                # ScalarE mult + DVE add -> acc_d
M_TAPS = 11               # DVE mult + Pool add -> acc_p
assert P_TAPS + D_FULL + A_TAPS + M_TAPS == 49

BF16 = ml_dtypes.bfloat16

_CACHE = {}


def _build_nc(repeat=1):
    import concourse.bacc as bacc
    import concourse.tile as tile
    from concourse import mybir

    f32 = mybir.dt.float32
    bf16 = mybir.dt.bfloat16

    nc = bacc.Bacc()

    xk = nc.dram_tensor("xk", [NB, CIN, HK, HK], bf16, kind="ExternalInput")
    xs = nc.dram_tensor("xs", [NB, CIN, HS, HS], bf16, kind="ExternalInput")
    wkT = nc.dram_tensor("wkT", [CIN, HID], bf16, kind="ExternalInput")
    wsT = nc.dram_tensor("wsT", [CIN, HID], bf16, kind="ExternalInput")
    wh1T = nc.dram_tensor("wh1T", [HID, HID], bf16, kind="ExternalInput")
    wh2T = nc.dram_tensor("wh2T", [HID, COUT], bf16, kind="ExternalInput")
    # packed per-channel scalars: cols = sck,shk,scs,shs,sch,shh,bh2(pad)
    scl = nc.dram_tensor("scl", [HID, 8], f32, kind="ExternalInput")
    ident = nc.dram_tensor("ident", [128, 128], bf16, kind="ExternalInput")
    out = nc.dram_tensor("out", [NB, COUT, HO, HO], f32, kind="ExternalOutput")

    relu = mybir.ActivationFunctionType.Relu
    idfn = mybir.ActivationFunctionType.Identity
    copyfn = mybir.ActivationFunctionType.Copy
    mult = mybir.AluOpType.mult
    addop = mybir.AluOpType.add
    iseq = mybir.AluOpType.is_equal

    with tile.TileContext(nc) as tc:
        with (
            tc.tile_pool(name="const", bufs=1) as cpool,
            tc.tile_pool(name="act", bufs=1) as apool,
            tc.tile_pool(name="stream", bufs=2) as spool,
            tc.tile_pool(name="diagp", bufs=3 * P_TAPS) as dpool,
            tc.tile_pool(name="psum", bufs=1, space="PSUM") as ppool,
        ):
            # ---- constants -------------------------------------------------
            # critical-path loads first: the kernel branch gates the xcorr
            # diag builds, so wk/scl/xk go ahead; ws next (search conv);
            # heads + ident later. Scale vectors come packed in one tensor.
            wk_t, ws_t, wh1_t, wh2_t = [], [], [], []
            scl_t = []
            for kc in range(NCH):
                sl = slice(kc * 128, (kc + 1) * 128)
                w1 = cpool.tile([128, HID], bf16, name=f"wk_{kc}")
                nc.sync.dma_start(w1[:], wkT[sl, :])
                wk_t.append(w1)
                sv = cpool.tile([128, 8], f32, name=f"scl_{kc}")
                nc.sync.dma_start(sv[:], scl[sl, :])
                scl_t.append(sv)
            xk_ap = xk[:].rearrange("b c h w -> c b (h w)")
            xk_t = []
            for kc in range(NCH):
                t = apool.tile([128, NB, HK * HK], bf16, name=f"xk_t{kc}")
                nc.sync.dma_start(t[:], xk_ap[kc * 128:(kc + 1) * 128])
                xk_t.append(t)
            for kc in range(NCH):
                sl = slice(kc * 128, (kc + 1) * 128)
                w2 = cpool.tile([128, HID], bf16, name=f"ws_{kc}")
                nc.sync.dma_start(w2[:], wsT[sl, :])
                ws_t.append(w2)
            for kc in range(NCH):
                sl = slice(kc * 128, (kc + 1) * 128)
                w3 = cpool.tile([128, HID], bf16, name=f"wh1_{kc}")
                nc.sync.dma_start(w3[:], wh1T[sl, :])
                wh1_t.append(w3)
                w4 = cpool.tile([128, COUT], bf16, name=f"wh2_{kc}")
                nc.sync.dma_start(w4[:], wh2T[sl, :])
                wh2_t.append(w4)
            id_t = cpool.tile([128, 128], bf16, name="id_t")
            nc.sync.dma_start(id_t[:], ident[:])
            sck_t = [sv[:, 0:1] for sv in scl_t]
            shk_t = [sv[:, 1:2] for sv in scl_t]
            scs_t = [sv[:, 2:3] for sv in scl_t]
            shs_t = [sv[:, 3:4] for sv in scl_t]
            sch_t = [sv[:, 4:5] for sv in scl_t]
            shh_t = [sv[:, 5:6] for sv in scl_t]
            bh2_t = scl_t[0][0:COUT, 6:7]

            # ---- kernel branch conv (all NB batches at once) ---------------
            # k_feat[mc]: [128, NB, 49] fp32 (source of per-partition scalars)
            k_feat = []
            for mc in range(NCH):
                kf = apool.tile([128, NB, HK * HK], f32, name=f"k_feat{mc}")
                for half in range(2):
                    bs = slice(half * (NB // 2), (half + 1) * (NB // 2))
                    ps = ppool.tile([128, NB // 2, HK * HK], f32,
                                    name="ps_cs", tag="csA", bufs=1)
                    for kc in range(NCH):
                        nc.tensor.matmul(
                            ps[:],
                            wk_t[kc][:, mc * 128:(mc + 1) * 128],
                            xk_t[kc][:, bs, :],
                            start=(kc == 0), stop=(kc == NCH - 1),
                        )
                    nc.scalar.activation(kf[:, bs, :], ps[:], relu,
                                         bias=shk_t[mc], scale=sck_t[mc])
                k_feat.append(kf)

            for _rep in range(repeat):
              # ---- main loop over batch groups -----------------------------
              NGRP = NB // GB
              xs_ap = xs[:].rearrange("b c h w -> c b (h w)")
              feat = {}
              ys = {}

              def tap_split(g, bl):
                  """(pe, dve, act, pool) tap counts; the final unit leans
                  on the PE so the vector engines drain earlier."""
                  if g == NGRP - 1 and bl == GB - 1:
                      return (37, 4, 4, 4)
                  return (P_TAPS, D_FULL, A_TAPS, M_TAPS)

              def emit_diags(it):
                  """diagonal weight tiles on Pool (affine_select)."""
                  g, bl, mc = it
                  b_abs = g * GB + bl
                  lst = []
                  for uv in range(tap_split(g, bl)[0]):
                      dg = dpool.tile([128, 128], bf16, name="dg", tag="diag")
                      kcol = k_feat[mc][:, b_abs, uv:uv + 1]
                      nc.gpsimd.affine_select(
                          dg[:], kcol.broadcast_to((128, 128)),
                          pattern=[[-1, 128]], compare_op=iseq,
                          fill=0.0, base=0, channel_multiplier=1)
                      lst.append(dg)
                  return lst

              def emit_group_conv(g):
                  """xs load + search conv + BN/ReLU for one batch group."""
                  gsl = slice(g * GB, (g + 1) * GB)
                  xs_t = []
                  for kc in range(NCH):
                      t = spool.tile([128, GB, HS * HS], bf16,
                                     name=f"xs_t{kc}", tag=f"xs{kc}")
                      # group 0 loads ride the (startup-idle) Act queue so
                      # they don't serialize behind the weight loads on SP
                      q = nc.scalar if g == 0 else nc.sync
                      q.dma_start(t[:],
                                  xs_ap[kc * 128:(kc + 1) * 128, gsl])
                      xs_t.append(t)
                  s_feat = []
                  for mc in range(NCH):
                      sf = spool.tile([128, GB, HS, HS], bf16,
                                      name=f"s_feat{mc}", tag=f"sf{mc}")
                      for bl in range(GB):
                          for ph, (r0, r1) in enumerate(((0, 16), (16, 31))):
                              ps = ppool.tile([128, r1 - r0, HS], f32,
                                              name="ps_cs2",
                                              tag=("csA" if ph == 0 else "csB"),
                                              bufs=1)
                              for kc in range(NCH):
                                  nc.tensor.matmul(
                                      ps[:],
                                      ws_t[kc][:, mc * 128:(mc + 1) * 128],
                                      xs_t[kc][:, bl, r0 * HS:r1 * HS],
                                      start=(kc == 0), stop=(kc == NCH - 1),
                                  )
                              nc.scalar.activation(sf[:, bl, r0:r1, :], ps[:],
                                                   relu, bias=shs_t[mc],
                                                   scale=scs_t[mc])
                      s_feat.append(sf)
                  return s_feat

              s_feat = emit_group_conv(0)
              for g in range(NGRP):
                  s_feat_next = None
                  for bl in range(GB):
                      if bl == GB - 2 and g + 1 < NGRP:
                          # hoist the next group's conv ahead of this
                          # group's last batch so ScalarE/DVE are not
                          # starved at the group boundary
                          s_feat_next = emit_group_conv(g + 1)
                      b_abs = g * GB + bl
                      n_pe, n_d, n_a, n_m = tap_split(g, bl)
                      for mc in range(NCH):
                          it = (g, bl, mc)
                          kcol = lambda uv: k_feat[mc][:, b_abs, uv:uv + 1]
                          swin = lambda uv, r0, r1: s_feat[mc][
                              :, bl, uv // 7 + r0:uv // 7 + r1,
                              uv % 7:uv % 7 + HO]
                          diags = emit_diags(it)

                          # --- vector-engine taps ---------------------------
                          # acc_d: n_d DVE taps + n_a ScalarE-mult taps
                          # acc_p: n_m DVE-mult/Pool-add taps
                          acc_d = apool.tile([128, HO, HO], bf16,
                                             name="acc_d", tag=f"accd{mc}",
                                             bufs=4)
                          acc_p = apool.tile([128, HO, HO], bf16,
                                             name="acc_p", tag=f"accp{mc}",
                                             bufs=4)
                          uv = n_pe
                          # seed acc_p (Pool's adds chain the longest; start
                          # its feed first)
                          nc.vector.tensor_scalar_mul(
                              acc_p[:], swin(uv, 0, HO), kcol(uv))
                          uv += 1
                          for i in range(n_m - 1):
                              tmp = apool.tile([128, HO, HO], bf16,
                                               name="tmpp", tag="tmpp", bufs=6)
                              nc.vector.tensor_scalar_mul(
                                  tmp[:], swin(uv, 0, HO), kcol(uv))
                              nc.gpsimd.tensor_tensor(
                                  acc_p[:], acc_p[:], tmp[:], op=addop)
                              uv += 1
                          # seed acc_d
                          nc.vector.tensor_scalar_mul(
                              acc_d[:], swin(uv, 0, HO), kcol(uv))
                          uv += 1
                          for i in range(n_d - 1):
                              tmp = apool.tile([128, HO, HO], bf16,
                                               name="tmp", tag="tmp", bufs=6)
                              nc.vector.tensor_scalar_mul(
                                  tmp[:], swin(uv, 0, HO), kcol(uv))
                              nc.vector.tensor_tensor(
                                  acc_d[:], acc_d[:], tmp[:], addop)
                              uv += 1
                          for i in range(n_a):
                              tmp = apool.tile([128, HO, HO], bf16,
                                               name="tmpa", tag="tmpa", bufs=6)
                              nc.scalar.activation(tmp[:], swin(uv, 0, HO),
                                                   copyfn, scale=kcol(uv))
                              nc.vector.tensor_tensor(
                                  acc_d[:], acc_d[:], tmp[:], addop)
                              uv += 1
                          assert uv == 49
                          # merge acc_d into acc_p on Pool
                          nc.gpsimd.tensor_tensor(
                              acc_p[:], acc_p[:], acc_d[:], op=addop)

                          ft = apool.tile([128, HO * HO], bf16,
                                          name=f"feat{mc}", tag=f"f{mc}",
                                          bufs=3)
                          # phase A: output rows 0:20 (N=500)
                          psA = ppool.tile([128, 20 * HO], f32,
                                           name="ps_xc", tag="xc", bufs=3)
                          for t in range(n_pe):
                              nc.tensor.matmul(
                                  psA[:], diags[t][:], swin(t, 0, 20),
                                  start=(t == 0), stop=False,
                              )
                          nc.tensor.matmul(
                              psA[:], id_t[:], acc_p[:, 0:20, :],
                              start=False, stop=True,
                          )
                          nc.scalar.activation(ft[:, 0:500], psA[:], copyfn)
                          # phase B: output rows 20:25 (N=125)
                          psB = ppool.tile([128, 5 * HO], f32,
                                           name="ps_xcB", tag="xc", bufs=3)
                          for t in range(n_pe):
                              nc.tensor.matmul(
                                  psB[:], diags[t][:], swin(t, 20, 25),
                                  start=(t == 0), stop=False,
                              )
                          nc.tensor.matmul(
                              psB[:], id_t[:], acc_p[:, 20:25, :],
                              start=False, stop=True,
                          )
                          nc.scalar.activation(ft[:, 500:625], psB[:], copyfn)
                          feat[mc] = ft

                          if mc < NCH - 1:
                              continue

                          # head conv1 + BN/ReLU
                          for mq in range(NCH):
                              yt = apool.tile([128, HO * HO], bf16,
                                              name=f"y{mq}", tag=f"y{mq}",
                                              bufs=2)
                              for ph, (c0, c1) in enumerate(((0, 500),
                                                            (500, 625))):
                                  ps = ppool.tile([128, c1 - c0], f32,
                                                  name="ps_hd",
                                                  tag=("hdA" if ph == 0
                                                       else "hdB"),
                                                  bufs=(2 if ph == 0 else 1))
                                  for kc in range(NCH):
                                      nc.tensor.matmul(
                                          ps[:],
                                          wh1_t[kc][:, mq * 128:(mq + 1) * 128],
                                          feat[kc][:, c0:c1],
                                          start=(kc == 0), stop=(kc == NCH - 1),
                                      )
                                  nc.scalar.activation(yt[:, c0:c1], ps[:],
                                                       relu, bias=shh_t[mq],
                                                       scale=sch_t[mq])
                              ys[mq] = yt

                          # head conv2 + bias
                          ot = apool.tile([COUT, HO * HO], f32,
                                          name="ot", tag="ot", bufs=2)
                          for ph, (c0, c1) in enumerate(((0, 500), (500, 625))):
                              ps = ppool.tile([COUT, c1 - c0], f32,
                                              name="ps_o",
                                              tag=("hdA" if ph == 0 else "hdB"),
                                              bufs=(2 if ph == 0 else 1))
                              for kc in range(NCH):
                                  nc.tensor.matmul(
                                      ps[:],
                                      wh2_t[kc][:],
                                      ys[kc][:, c0:c1],
                                      start=(kc == 0), stop=(kc == NCH - 1),
                                  )
                              nc.scalar.activation(ot[:, c0:c1], ps[:], idfn,
                                                   bias=bh2_t, scale=1.0)
                          nc.sync.dma_start(
                              out[b_abs][:].rearrange("o h w -> o (h w)"), ot[:])
                  if s_feat_next is not None:
                      s_feat = s_feat_next

    nc.compile()
    return nc


def _get_nc():
    if "nc" not in _CACHE:
        _CACHE["nc"] = _build_nc()
    return _CACHE["nc"]


def kernel(kernel, search, w_k, g_k, b_k, m_k, v_k, w_s, g_s, b_s, m_s, v_s,
           w_h1, g_h, b_h, m_h, v_h, w_h2, bias_h2):
    from concourse.bass_utils import run_bass_kernel_spmd

    def fold(g, b, m, v):
        sc = (g / np.sqrt(v + EPS)).astype(np.float32)
        sh = (b - m * sc).astype(np.float32)
        return sc.reshape(-1, 1), sh.reshape(-1, 1)

    kernel, search, w_k, w_s, w_h1, w_h2, bias_h2 = [
        np.asarray(a) for a in
        (kernel, search, w_k, w_s, w_h1, w_h2, bias_h2)]
    g_k, b_k, m_k, v_k = map(np.asarray, (g_k, b_k, m_k, v_k))
    g_s, b_s, m_s, v_s = map(np.asarray, (g_s, b_s, m_s, v_s))
    g_h, b_h, m_h, v_h = map(np.asarray, (g_h, b_h, m_h, v_h))

    sck, shk = fold(g_k, b_k, m_k, v_k)
    scs, shs = fold(g_s, b_s, m_s, v_s)
    sch, shh = fold(g_h, b_h, m_h, v_h)

    scl = np.zeros((HID, 8), dtype=np.float32)
    scl[:, 0:1], scl[:, 1:2] = sck, shk
    scl[:, 2:3], scl[:, 3:4] = scs, shs
    scl[:, 4:5], scl[:, 5:6] = sch, shh
    scl[:COUT, 6] = bias_h2.astype(np.float32).ravel()
    common = {
        "wkT": np.ascontiguousarray(w_k.T).astype(BF16),
        "wsT": np.ascontiguousarray(w_s.T).astype(BF16),
        "wh1T": np.ascontiguousarray(w_h1.T).astype(BF16),
        "wh2T": np.ascontiguousarray(w_h2.T).astype(BF16),
        "scl": scl,
        "ident": np.eye(128, dtype=BF16),
    }
    xk_all = kernel.astype(BF16)
    xs_all = search.astype(BF16)

    in_maps = []
    for i in range(NCORES):
        bs = slice(i * NB, (i + 1) * NB)
        m = dict(common)
        m["xk"] = np.ascontiguousarray(xk_all[bs])
        m["xs"] = np.ascontiguousarray(xs_all[bs])
        in_maps.append(m)

    nc = _get_nc()
    res = run_bass_kernel_spmd(nc, in_maps, core_ids=list(range(NCORES)))
    return np.concatenate([res.results[i]["out"] for i in range(NCORES)],
                          axis=0)


# revision 15
# speedup vs baseline: 1.0915x; 1.0188x over previous
"""Trainium2 Bass kernel for DepthwiseXCorr (SiamRPN-style head).

Pipeline per batch sample:
  k = BN+ReLU(conv1x1(kernel, w_k))      [256, 7, 7]
  s = BN+ReLU(conv1x1(search, w_s))      [256, 31, 31]
  feat = depthwise_xcorr(s, k)           [256, 25, 25]
  y = BN+ReLU(conv1x1(feat, w_h1))
  out = conv1x1(y, w_h2) + bias          [20, 25, 25]

Sharding: pure data-parallel, batch 128 -> 16 per core across 8 cores.

Implementation notes:
- conv1x1 = matmul with channels on the contraction (partition) dim.
- depthwise xcorr: the 49 kernel taps (u, v) are split across FOUR engines:
  * P_TAPS on the PE as diag(k[:, u, v]) @ s[:, i+u, j+v] matmuls
    accumulating in PSUM; the diagonal weight tiles are built by the Pool
    (GPSIMD) engine with affine_select (iota p-j == 0 selects a stride-0
    broadcast of the per-channel scalar), which is far cheaper than
    scaling a 128x128 identity on DVE/ScalarE.
  * D_FULL taps run fully on VectorE (4x-mode tensor_scalar mult +
    2x-mode tensor_tensor add into a bf16 accumulator acc_d).
  * A_TAPS taps: ScalarE does the multiply (activation Copy with
    per-partition scale), VectorE adds into acc_d.
  * M_TAPS taps: VectorE does the multiply, Pool adds into acc_p.
  Pool then merges acc_d into acc_p, and one identity-weight matmul per
  PSUM phase folds acc_p into the PSUM accumulation group.
- Shifted search windows are plain strided APs into the search feature
  tile - no data movement.
- All matmuls in bf16 (PE full column rate), accumulation in fp32 PSUM,
  BN+ReLU epilogues on ScalarE with fp32 math, fp32 output.
"""

import sys

if "/opt/trn_rl_repo" not in sys.path:
    sys.path.insert(0, "/opt/trn_rl_repo")

import ml_dtypes
import numpy as np

B, CIN, HID, COUT = 128, 256, 256, 20
NCORES = 8
NB = B // NCORES          # batches per core
HS = 31                   # search spatial
HK = 7                    # kernel spatial
HO = HS - HK + 1          # 25, xcorr output spatial
EPS = 1e-5
GB = 4                    # batch group size for the search-branch pipeline
NCH = 2                   # channel chunks of 128

# xcorr tap assignment (49 taps total)
P_TAPS = 25               # PE diag matmuls (diags built on Pool)
D_FULL = 6                # DVE mult + DVE add -> acc_d
A_TAPS = 7                # ScalarE mult + DVE add -> acc_d
M_TAPS = 11               # DVE mult + Pool add -> acc_p
assert P_TAPS + D_FULL + A_TAPS + M_TAPS == 49

BF16 = ml_dtypes.bfloat16

_CACHE = {}


def _build_nc(repeat=1):
    import concourse.bacc as bacc
    import concourse.tile as tile
    from concourse import mybir

    f32 = mybir.dt.float32
    bf16 = mybir.dt.bfloat16

    nc = bacc.Bacc()

    xk = nc.dram_tensor("xk", [NB, CIN, HK, HK], bf16, kind="ExternalInput")
    xs = nc.dram_tensor("xs", [NB, CIN, HS, HS], bf16, kind="ExternalInput")
    wkT = nc.dram_tensor("wkT", [CIN, HID], bf16, kind="ExternalInput")
    wsT = nc.dram_tensor("wsT", [CIN, HID], bf16, kind="ExternalInput")
    wh1T = nc.dram_tensor("wh1T", [HID, HID], bf16, kind="ExternalInput")
    wh2T = nc.dram_tensor("wh2T", [HID, COUT], bf16, kind="ExternalInput")
    # packed per-channel scalars: cols = sck,shk,scs,shs,sch,shh,bh2(pad)
    scl = nc.dram_tensor("scl", [HID, 8], f32, kind="ExternalInput")
    ident = nc.dram_tensor("ident", [128, 128], bf16, kind="ExternalInput")
    out = nc.dram_tensor("out", [NB, COUT, HO, HO], f32, kind="ExternalOutput")

    relu = mybir.ActivationFunctionType.Relu
    idfn = mybir.ActivationFunctionType.Identity
    copyfn = mybir.ActivationFunctionType.Copy
    mult = mybir.AluOpType.mult
    addop = mybir.AluOpType.add
    iseq = mybir.AluOpType.is_equal

    with tile.TileContext(nc) as tc:
        with (
            tc.tile_pool(name="const", bufs=1) as cpool,
            tc.tile_pool(name="act", bufs=1) as apool,
            tc.tile_pool(name="stream", bufs=2) as spool,
            tc.tile_pool(name="diagp", bufs=3 * P_TAPS) as dpool,
            tc.tile_pool(name="psum", bufs=1, space="PSUM") as ppool,
        ):
            # ---- constants -------------------------------------------------
            # critical-path loads first: the kernel branch gates the xcorr
            # diag builds, so wk/scl/xk go ahead; ws next (search conv);
            # heads + ident later. Scale vectors come packed in one tensor.
            wk_t, ws_t, wh1_t, wh2_t = [], [], [], []
            scl_t = []
            for kc in range(NCH):
                sl = slice(kc * 128, (kc + 1) * 128)
                w1 = cpool.tile([128, HID], bf16, name=f"wk_{kc}")
                nc.sync.dma_start(w1[:], wkT[sl, :])
                wk_t.append(w1)
            xk_ap = xk[:].rearrange("b c h w -> c b (h w)")
            xk_t = []
            for kc in range(NCH):
                t = apool.tile([128, NB, HK * HK], bf16, name=f"xk_t{kc}")
                nc.sync.dma_start(t[:], xk_ap[kc * 128:(kc + 1) * 128])
                xk_t.append(t)
            for kc in range(NCH):
                sl = slice(kc * 128, (kc + 1) * 128)
                w2 = cpool.tile([128, HID], bf16, name=f"ws_{kc}")
                nc.sync.dma_start(w2[:], wsT[sl, :])
                ws_t.append(w2)
                sv = cpool.tile([128, 8], f32, name=f"scl_{kc}")
                nc.sync.dma_start(sv[:], scl[sl, :])
                scl_t.append(sv)
            for kc in range(NCH):
                sl = slice(kc * 128, (kc + 1) * 128)
                w3 = cpool.tile([128, HID], bf16, name=f"wh1_{kc}")
                nc.sync.dma_start(w3[:], wh1T[sl, :])
                wh1_t.append(w3)
                w4 = cpool.tile([128, COUT], bf16, name=f"wh2_{kc}")
                nc.sync.dma_start(w4[:], wh2T[sl, :])
                wh2_t.append(w4)
            id_t = cpool.tile([128, 128], bf16, name="id_t")
            nc.sync.dma_start(id_t[:], ident[:])
            sck_t = [sv[:, 0:1] for sv in scl_t]
            shk_t = [sv[:, 1:2] for sv in scl_t]
            scs_t = [sv[:, 2:3] for sv in scl_t]
            shs_t = [sv[:, 3:4] for sv in scl_t]
            sch_t = [sv[:, 4:5] for sv in scl_t]
            shh_t = [sv[:, 5:6] for sv in scl_t]
            bh2_t = scl_t[0][0:COUT, 6:7]

            # ---- kernel branch conv (all NB batches at once) ---------------
            # k_feat[mc]: [128, NB, 49] fp32 (source of per-partition scalars)
            k_feat = []
            for mc in range(NCH):
                kf = apool.tile([128, NB, HK * HK], f32, name=f"k_feat{mc}")
                for half in range(2):
                    bs = slice(half * (NB // 2), (half + 1) * (NB // 2))
                    ps = ppool.tile([128, NB // 2, HK * HK], f32,
                                    name="ps_cs", tag="csA", bufs=1)
                    for kc in range(NCH):
                        nc.tensor.matmul(
                            ps[:],
                            wk_t[kc][:, mc * 128:(mc + 1) * 128],
                            xk_t[kc][:, bs, :],
                            start=(kc == 0), stop=(kc == NCH - 1),
                        )
                    nc.scalar.activation(kf[:, bs, :], ps[:], relu,
                                         bias=shk_t[mc], scale=sck_t[mc])
                k_feat.append(kf)

            for _rep in range(repeat):
              # ---- main loop over batch groups -----------------------------
              NGRP = NB // GB
              xs_ap = xs[:].rearrange("b c h w -> c b (h w)")
              feat = {}
              ys = {}

              def tap_split(g, bl):
                  """(pe, dve, act, pool) tap counts; the final unit leans
                  on the PE so the vector engines drain earlier."""
                  if g == NGRP - 1 and bl == GB - 1:
                      return (31, 5, 5, 8)
                  return (P_TAPS, D_FULL, A_TAPS, M_TAPS)

              def emit_diags(it):
                  """diagonal weight tiles on Pool (affine_select)."""
                  g, bl, mc = it
                  b_abs = g * GB + bl
                  lst = []
                  for uv in range(tap_split(g, bl)[0]):
                      dg = dpool.tile([128, 128], bf16, name="dg", tag="diag")
                      kcol = k_feat[mc][:, b_abs, uv:uv + 1]
                      nc.gpsimd.affine_select(
                          dg[:], kcol.broadcast_to((128, 128)),
                          pattern=[[-1, 128]], compare_op=iseq,
                          fill=0.0, base=0, channel_multiplier=1)
                      lst.append(dg)
                  return lst

              def emit_group_conv(g):
                  """xs load + search conv + BN/ReLU for one batch group."""
                  gsl = slice(g * GB, (g + 1) * GB)
                  xs_t = []
                  for kc in range(NCH):
                      t = spool.tile([128, GB, HS * HS], bf16,
                                     name=f"xs_t{kc}", tag=f"xs{kc}")
                      # group 0 loads ride the (startup-idle) Act/Pool
                      # queues so they don't serialize behind the weight
                      # loads on SP
                      q = (nc.scalar if kc == 0 else nc.gpsimd) \
                          if g == 0 else nc.sync
                      q.dma_start(t[:],
                                  xs_ap[kc * 128:(kc + 1) * 128, gsl])
                      xs_t.append(t)
                  s_feat = []
                  for mc in range(NCH):
                      sf = spool.tile([128, GB, HS, HS], bf16,
                                      name=f"s_feat{mc}", tag=f"sf{mc}")
                      for bl in range(GB):
                          for ph, (r0, r1) in enumerate(((0, 16), (16, 31))):
                              ps = ppool.tile([128, r1 - r0, HS], f32,
                                              name="ps_cs2",
                                              tag=("csA" if ph == 0 else "csB"),
                                              bufs=1)
                              for kc in range(NCH):
                                  nc.tensor.matmul(
                                      ps[:],
                                      ws_t[kc][:, mc * 128:(mc + 1) * 128],
                                      xs_t[kc][:, bl, r0 * HS:r1 * HS],
                                      start=(kc == 0), stop=(kc == NCH - 1),
                                  )
                              nc.scalar.activation(sf[:, bl, r0:r1, :], ps[:],
                                                   relu, bias=shs_t[mc],
                                                   scale=scs_t[mc])
                      s_feat.append(sf)
                  return s_feat

              s_feat = emit_group_conv(0)
              for g in range(NGRP):
                  s_feat_next = None
                  for bl in range(GB):
                      if bl == GB - 2 and g + 1 < NGRP:
                          # hoist the next group's conv ahead of this
                          # group's last batch so ScalarE/DVE are not
                          # starved at the group boundary
                          s_feat_next = emit_group_conv(g + 1)
                      b_abs = g * GB + bl
                      n_pe, n_d, n_a, n_m = tap_split(g, bl)
                      for mc in range(NCH):
                          it = (g, bl, mc)
                          kcol = lambda uv: k_feat[mc][:, b_abs, uv:uv + 1]
                          swin = lambda uv, r0, r1: s_feat[mc][
                              :, bl, uv // 7 + r0:uv // 7 + r1,
                              uv % 7:uv % 7 + HO]
                          diags = emit_diags(it)

                          # --- vector-engine taps ---------------------------
                          # acc_d: n_d DVE taps + n_a ScalarE-mult taps
                          # acc_p: n_m DVE-mult/Pool-add taps
                          acc_d = apool.tile([128, HO, HO], bf16,
                                             name="acc_d", tag=f"accd{mc}",
                                             bufs=4)
                          acc_p = apool.tile([128, HO, HO], bf16,
                                             name="acc_p", tag=f"accp{mc}",
                                             bufs=4)
                          uv = n_pe
                          # seed acc_p (Pool's adds chain the longest; start
                          # its feed first)
                          nc.vector.tensor_scalar_mul(
                              acc_p[:], swin(uv, 0, HO), kcol(uv))
                          uv += 1
                          for i in range(n_m - 1):
                              tmp = apool.tile([128, HO, HO], bf16,
                                               name="tmpp", tag="tmpp", bufs=6)
                              nc.vector.tensor_scalar_mul(
                                  tmp[:], swin(uv, 0, HO), kcol(uv))
                              nc.gpsimd.tensor_tensor(
                                  acc_p[:], acc_p[:], tmp[:], op=addop)
                              uv += 1
                          # seed acc_d
                          nc.vector.tensor_scalar_mul(
                              acc_d[:], swin(uv, 0, HO), kcol(uv))
                          uv += 1
                          for i in range(n_d - 1):
                              tmp = apool.tile([128, HO, HO], bf16,
                                               name="tmp", tag="tmp", bufs=6)
                              nc.vector.tensor_scalar_mul(
                                  tmp[:], swin(uv, 0, HO), kcol(uv))
                              nc.vector.tensor_tensor(
                                  acc_d[:], acc_d[:], tmp[:], addop)
                              uv += 1
                          for i in range(n_a):
                              tmp = apool.tile([128, HO, HO], bf16,
                                               name="tmpa", tag="tmpa", bufs=6)
                              nc.scalar.activation(tmp[:], swin(uv, 0, HO),
                                                   copyfn, scale=kcol(uv))
                              nc.vector.tensor_tensor(
                                  acc_d[:], acc_d[:], tmp[:], addop)
                              uv += 1
                          assert uv == 49
                          # merge acc_d into acc_p on Pool
                          nc.gpsimd.tensor_tensor(
                              acc_p[:], acc_p[:], acc_d[:], op=addop)

                          # feat padded to 626 cols so the head convs can
                          # run two equal 313-wide, bank-aligned phases and
                          # finish with ONE epilogue op each
                          ft = apool.tile([128, HO * HO + 1], bf16,
                                          name=f"feat{mc}", tag=f"f{mc}",
                                          bufs=3)
                          nc.gpsimd.memset(ft[:, 625:626], 0.0)
                          # phase A: output rows 0:20 (N=500)
                          psA = ppool.tile([128, 20 * HO], f32,
                                           name="ps_xc", tag="xc", bufs=4)
                          for t in range(n_pe):
                              nc.tensor.matmul(
                                  psA[:], diags[t][:], swin(t, 0, 20),
                                  start=(t == 0), stop=False,
                              )
                          nc.tensor.matmul(
                              psA[:], id_t[:], acc_p[:, 0:20, :],
                              start=False, stop=True,
                          )
                          nc.scalar.activation(ft[:, 0:500], psA[:], copyfn)
                          # phase B: output rows 20:25 (N=125)
                          psB = ppool.tile([128, 5 * HO], f32,
                                           name="ps_xcB", tag="xc", bufs=4)
                          for t in range(n_pe):
                              nc.tensor.matmul(
                                  psB[:], diags[t][:], swin(t, 20, 25),
                                  start=(t == 0), stop=False,
                              )
                          nc.tensor.matmul(
                              psB[:], id_t[:], acc_p[:, 20:25, :],
                              start=False, stop=True,
                          )
                          nc.scalar.activation(ft[:, 500:625], psB[:], copyfn)
                          feat[mc] = ft

                          if mc < NCH - 1:
                              continue

                          # head conv1 + BN/ReLU: two 313-wide phases into
                          # one 2-bank psum (offsets 0 / 512), one epilogue
                          for mq in range(NCH):
                              yt = apool.tile([128, HO * HO + 1], bf16,
                                              name=f"y{mq}", tag=f"y{mq}",
                                              bufs=2)
                              ps = ppool.tile([128, 2, 512], f32,
                                              name="ps_hd", tag="hd", bufs=1)
                              for ph in range(2):
                                  for kc in range(NCH):
                                      nc.tensor.matmul(
                                          ps[:, ph, 0:313],
                                          wh1_t[kc][:, mq * 128:(mq + 1) * 128],
                                          feat[kc][:, ph * 313:(ph + 1) * 313],
                                          start=(kc == 0), stop=(kc == NCH - 1),
                                      )
                              nc.scalar.activation(
                                  yt[:].rearrange("p (a b) -> p a b", a=2),
                                  ps[:, :, 0:313],
                                  relu, bias=shh_t[mq], scale=sch_t[mq])
                              ys[mq] = yt

                          # head conv2 + bias (same phase scheme)
                          ot = apool.tile([COUT, HO * HO + 1], f32,
                                          name="ot", tag="ot", bufs=2)
                          ps = ppool.tile([COUT, 2, 512], f32,
                                          name="ps_o", tag="hd", bufs=1)
                          for ph in range(2):
                              for kc in range(NCH):
                                  nc.tensor.matmul(
                                      ps[:, ph, 0:313],
                                      wh2_t[kc][:],
                                      ys[kc][:, ph * 313:(ph + 1) * 313],
                                      start=(kc == 0), stop=(kc == NCH - 1),
                                  )
                          nc.scalar.activation(
                              ot[:].rearrange("p (a b) -> p a b", a=2),
                              ps[:, :, 0:313], idfn, bias=bh2_t, scale=1.0)
                          nc.sync.dma_start(
                              out[b_abs][:].rearrange("o h w -> o (h w)"),
                              ot[:, 0:625])
                  if s_feat_next is not None:
                      s_feat = s_feat_next

    nc.compile()
    return nc


def _get_nc():
    if "nc" not in _CACHE:
        _CACHE["nc"] = _build_nc()
    return _CACHE["nc"]


def kernel(kernel, search, w_k, g_k, b_k, m_k, v_k, w_s, g_s, b_s, m_s, v_s,
           w_h1, g_h, b_h, m_h, v_h, w_h2, bias_h2):
    from concourse.bass_utils import run_bass_kernel_spmd

    def fold(g, b, m, v):
        sc = (g / np.sqrt(v + EPS)).astype(np.float32)
        sh = (b - m * sc).astype(np.float32)
        return sc.reshape(-1, 1), sh.reshape(-1, 1)

    kernel, search, w_k, w_s, w_h1, w_h2, bias_h2 = [
        np.asarray(a) for a in
        (kernel, search, w_k, w_s, w_h1, w_h2, bias_h2)]
    g_k, b_k, m_k, v_k = map(np.asarray, (g_k, b_k, m_k, v_k))
    g_s, b_s, m_s, v_s = map(np.asarray, (g_s, b_s, m_s, v_s))
    g_h, b_h, m_h, v_h = map(np.asarray, (g_h, b_h, m_h, v_h))

    sck, shk = fold(g_k, b_k, m_k, v_k)
    scs, shs = fold(g_s, b_s, m_s, v_s)
    sch, shh = fold(g_h, b_h, m_h, v_h)

    scl = np.zeros((HID, 8), dtype=np.float32)
    scl[:, 0:1], scl[:, 1:2] = sck, shk
    scl[:, 2:3], scl[:, 3:4] = scs, shs
    scl[:, 4:5], scl[:, 5:6] = sch, shh
    scl[:COUT, 6] = bias_h2.astype(np.float32).ravel()
    common = {
        "wkT": np.ascontiguousarray(w_k.T).astype(BF16),
        "wsT": np.ascontiguousarray(w_s.T).astype(BF16),
        "wh1T": np.ascontiguousarray(w_h1.T).astype(BF16),
        "wh2T": np.ascontiguousarray(w_h2.T).astype(BF16),
        "scl": scl,
        "ident": np.eye(128, dtype=BF16),
    }
    xk_all = kernel.astype(BF16)
    xs_all = search.astype(BF16)

    in_maps = []
    for i in range(NCORES):
        bs = slice(i * NB, (i + 1) * NB)
        m = dict(common)
        m["xk"] = np.ascontiguousarray(xk_all[bs])
        m["xs"] = np.ascontiguousarray(xs_all[bs])
        in_maps.append(m)

    nc = _get_nc()
    res = run_bass_kernel_spmd(nc, in_maps, core_ids=list(range(NCORES)))
    return np.concatenate([res.results[i]["out"] for i in range(NCORES)],
                          axis=0)
